# revision 1
# baseline (speedup 1.0000x reference)
"""MoE FFN (E=8 experts, top-2) — expert-parallel Bass/Tile kernel for 8 TRN2 cores.

Strategy:
  - Host computes the (tiny) router: logits = x @ gate_w.T, top-2 per token,
    renormalized weights (= sigmoid of logit differences).  This is the
    sharding decision: token n is dispatched to cores e1(n), e2(n).
  - Core e receives its expert's weights (pre-tiled) and the gathered,
    transposed tokens XgT [D, C] plus per-token gate weights.
  - Device: hT = gelu(w1.T @ xgT + b1)  (feature-major), then
    Y = hT.T @ w2 scaled by gate (fused in PSUM eviction).  Matmuls run as
    float32r (full-rate fp32, ~11-bit mantissa; PE rounds operands itself).
  - Host combine: out[idx_e] += Yg_e (each token appears in exactly 2 experts,
    never twice in one), plus the gate-weighted b2 term.
"""

import re

import numpy as np

import bass_rust
import concourse.bass as bass
import concourse.mybir as mybir
import concourse.tile as tile
from concourse import bacc, bass_utils

P = 128
D_MODEL = 1024
D_HID = 4096
E = 8
TOP_K = 2
N_CORES = 8

DC = D_MODEL // P          # 8 d-chunks (contraction for mm1)
HC = D_HID // P            # 32 h-chunks
HG = 4                     # h-chunks per w1 load group
NHG = HC // HG             # 8 groups
C = 1152                   # per-expert token capacity (>= max expert load)
# chunk-aligned token blocks (start, size); mm1 subtile sizes stay >=256 so
# float32r runs at full rate, and <=512 for one fp32 PSUM bank
BLOCKS = [(0, 640, (320, 320)), (640, 512, (512,))]
DT = 2                     # output d tiles
DTS = D_MODEL // DT        # 512
MAX_BT = max(bt for _, bt, _ in BLOCKS)

F32 = mybir.dt.float32
F32R = mybir.dt.float32r
MM_DT = F32R               # matmul operand dtype (F32R = fast, F32 = exact)


_tail_patched = False


def _patch_light_tail():
    """Replace Tile's end-of-context machinery (multi-wait drain + two
    all-engine EVSEM barriers + semaphore range-clears, ~10us on HW) with
    single-wait drains on the sync engine covering every logical proc's final
    tick.  The NEFF is executed once per load in this flow, so semaphores
    need not be recycled."""
    global _tail_patched
    if _tail_patched:
        return
    _tail_patched = True

    def _drain_and_barrier(self, tick_clock, wait_clock):
        gc = tick_clock.global_clock
        ticks = eval(re.match(r"VectorClock\((.*)\)", repr(gc)).group(1))
        n = len(ticks)
        for i, v in enumerate(ticks):
            if v > 0:
                vc = bass_rust.VectorClock(
                    [v if j == i else 0 for j in range(n)])
                w = self.nc.sync.drain()
                wait_clock.add_sem_waits(
                    w.ins,
                    bass_rust.ScopedClock({None: vc}),
                    bass_rust.ScopedClock({}),
                )
        popped = self.nc._tile_sem_poison_stack.pop()
        assert popped is self._sem_poison

    tile.TileContext._drain_and_barrier = _drain_and_barrier


def build_nc():
    _patch_light_tail()
    nc = bacc.Bacc("TRN2", target_bir_lowering=False, debug=False,
                   num_devices=N_CORES)

    # Inputs, pre-tiled on host into consumption order (all contiguous DMAs):
    #   xgt  [DC, P, C]         xgt[dc, p, n] = Xg[n, dc*128+p]
    #   w1t  [NHG, DC, P, HG*P] w1t[hg, dc, p, k*128+j] = w1[dc*128+p, (hg*4+k)*128+j]
    #   w2t  [DT, HC, P, DTS]   w2t[dt, hc, p, j] = w2[hc*128+p, dt*512+j]
    #   b1t  [P, HC]            b1t[p, hc] = b1[hc*128+p]
    #   gt   [P, C//P]          gt[p, c] = gate[c*128+p]
    xgt = nc.dram_tensor("xgt", [DC, P, C], MM_DT, kind="ExternalInput")
    w1t = nc.dram_tensor("w1t", [NHG, DC, P, HG * P], MM_DT, kind="ExternalInput")
    w2t = nc.dram_tensor("w2t", [DT, HC, P, DTS], MM_DT, kind="ExternalInput")
    b1t = nc.dram_tensor("b1t", [P, HC], F32, kind="ExternalInput")
    gt = nc.dram_tensor("gt", [P, C // P], F32, kind="ExternalInput")
    yg = nc.dram_tensor("yg", [C, D_MODEL], F32, kind="ExternalOutput")

    with tile.TileContext(nc) as tc:
        with (
            tc.tile_pool(name="const", bufs=1) as const,
            tc.tile_pool(name="xg", bufs=1) as xg_pool,
            tc.tile_pool(name="w1", bufs=16) as w1_pool,
            tc.tile_pool(name="w2", bufs=10) as w2_pool,
            tc.tile_pool(name="ht", bufs=HC + 4) as ht_pool,
            tc.tile_pool(name="yo", bufs=4) as yo_pool,
            tc.tile_pool(name="ps1", bufs=3, space="PSUM") as ps1,
            tc.tile_pool(name="ps2", bufs=5, space="PSUM") as ps2,
        ):
            b1_sb = const.tile([P, HC], F32, name="b1sb")
            nc.sync.dma_start(out=b1_sb[:], in_=b1t[:, :])
            g_sb = const.tile([P, C // P], F32, name="gsb")
            nc.sync.dma_start(out=g_sb[:], in_=gt[:, :])

            # xg tiles per (block, d-chunk); DMAs alternate between the two
            # HWDGE rings (SP and ACT) to double head bandwidth.  Rings are
            # FIFO, so enqueue the first matmul chain's inputs (w1 group 0
            # interleaved with xg block 0, in d-chunk order) ahead of the bulk.
            first_w1 = {}
            xg_sb = {}
            xg_parts = {}
            b0_n0, b0_bt, b0_subs = BLOCKS[0]
            half = b0_subs[0]
            for dc in range(DC):
                w1_sb = w1_pool.tile([P, HG * P], MM_DT, name="w1sb")
                eng = nc.sync if dc % 2 == 0 else nc.scalar
                eng.dma_start(out=w1_sb[:], in_=w1t[0, dc, :, :])
                first_w1[(0, dc)] = w1_sb
                t = xg_pool.tile([P, b0_bt], MM_DT, name=f"xg{b0_n0}_{dc}")
                eng2 = nc.sync if dc % 2 == 1 else nc.scalar
                eng2.dma_start(out=t[:, :half],
                               in_=xgt[dc, :, b0_n0:b0_n0 + half])
                xg_sb[(b0_n0, dc)] = t
                xg_parts[dc] = t
            for dc in range(DC):
                eng2 = nc.sync if dc % 2 == 1 else nc.scalar
                eng2.dma_start(out=xg_parts[dc][:, half:],
                               in_=xgt[dc, :, b0_n0 + half:b0_n0 + b0_bt])
            for n0, BT, _ in BLOCKS[1:]:
                for dc in range(DC):
                    t = xg_pool.tile([P, BT], MM_DT, name=f"xg{n0}_{dc}")
                    eng = nc.sync if dc % 2 == 0 else nc.scalar
                    eng.dma_start(out=t[:], in_=xgt[dc, :, n0:n0 + BT])
                    xg_sb[(n0, dc)] = t

            def evict(pss, n0, dt, NCH):
                for ncq in range(NCH):
                    yo = yo_pool.tile([P, DTS], F32, name="yo")
                    gcol = (n0 + ncq * P) // P
                    if ncq % 2 == 0:
                        nc.vector.tensor_scalar_mul(
                            yo[:], pss[ncq][:], g_sb[:, gcol:gcol + 1])
                    else:
                        nc.scalar.activation(
                            yo[:], pss[ncq][:],
                            mybir.ActivationFunctionType.Copy,
                            scale=g_sb[:, gcol:gcol + 1])
                    half_d = DTS // 2
                    row0 = n0 + ncq * P
                    nc.sync.dma_start(
                        out=yg[row0:row0 + P, dt * DTS:dt * DTS + half_d],
                        in_=yo[:, :half_d],
                    )
                    nc.scalar.dma_start(
                        out=yg[row0:row0 + P,
                               dt * DTS + half_d:(dt + 1) * DTS],
                        in_=yo[:, half_d:],
                    )

            for bi, (n0, BT, SUBS) in enumerate(BLOCKS):
                NCH = BT // P
                # ---- mm1: hT[hc] = gelu(w1.T @ xgT + b1) ----
                ht_tiles = []
                w1_cache = dict(first_w1) if bi == 0 else {}
                for hc in range(HC):
                    hg, k = divmod(hc, HG)
                    ht = ht_pool.tile([P, MAX_BT], MM_DT, name="ht")
                    sub0 = 0
                    for SUB in SUBS:
                        ps = ps1.tile([P, SUB], F32, name="ps1")
                        for dc in range(DC):
                            if (hg, dc) not in w1_cache:
                                w1_sb = w1_pool.tile([P, HG * P], MM_DT,
                                                     name="w1sb")
                                eng = nc.sync if (hg + dc) % 2 == 0 else nc.scalar
                                eng.dma_start(out=w1_sb[:],
                                              in_=w1t[hg, dc, :, :])
                                w1_cache[(hg, dc)] = w1_sb
                            w1_sb = w1_cache[(hg, dc)]
                            nc.tensor.matmul(
                                ps[:],
                                lhsT=w1_sb[:, k * P:(k + 1) * P],
                                rhs=xg_sb[(n0, dc)][:, sub0:sub0 + SUB],
                                start=(dc == 0),
                                stop=(dc == DC - 1),
                            )
                        nc.scalar.activation(
                            ht[:, sub0:sub0 + SUB], ps[:],
                            mybir.ActivationFunctionType.Gelu,
                            bias=b1_sb[:, hc:hc + 1],
                        )
                        sub0 += SUB
                    ht_tiles.append(ht)

                # ---- mm2: Y[n0:n0+BT] = (hT.T @ w2) * gate ----
                for dt in range(DT):
                    pss = [ps2.tile([P, DTS], F32, name="ps2")
                           for _ in range(NCH)]
                    for hc in range(HC):
                        w2_sb = w2_pool.tile([P, DTS], MM_DT, name="w2sb")
                        eng = nc.sync if hc % 2 == 0 else nc.scalar
                        eng.dma_start(out=w2_sb[:], in_=w2t[dt, hc, :, :])
                        for ncq in range(NCH):
                            nc.tensor.matmul(
                                pss[ncq][:],
                                lhsT=ht_tiles[hc][:, ncq * P:(ncq + 1) * P],
                                rhs=w2_sb[:],
                                start=(hc == 0),
                                stop=(hc == HC - 1),
                            )
                    evict(pss, n0, dt, NCH)
    nc.compile()
    return nc


_NC_CACHE = None
TRACE = False
LAST_RESULTS = None


def _get_nc():
    global _NC_CACHE
    if _NC_CACHE is None:
        _NC_CACHE = build_nc()
    return _NC_CACHE


def kernel(x, gate_w, w1, b1, w2, b2):
    x = np.asarray(x, dtype=np.float32)
    gate_w = np.asarray(gate_w, dtype=np.float32)
    w1 = np.asarray(w1, dtype=np.float32)
    b1 = np.asarray(b1, dtype=np.float32)
    w2 = np.asarray(w2, dtype=np.float32)
    b2 = np.asarray(b2, dtype=np.float32)

    B, T, D = x.shape
    N = B * T
    xf = x.reshape(N, D)

    # ---- router (host; 0.05% of model FLOPs — this is the sharding step) ----
    logits = xf @ gate_w.T                           # [N, E]
    order = np.argsort(-logits, axis=1, kind="stable")
    i1, i2 = order[:, 0], order[:, 1]
    l1 = logits[np.arange(N), i1].astype(np.float64)
    l2 = logits[np.arange(N), i2].astype(np.float64)
    g1 = (1.0 / (1.0 + np.exp(l2 - l1))).astype(np.float32)
    g2 = (1.0 - g1).astype(np.float32)

    # ---- dispatch: gather per-expert tokens, pre-tile all inputs ----
    in_maps = []
    idx_per_e = []
    for e in range(E):
        sel1 = np.nonzero(i1 == e)[0]
        sel2 = np.nonzero(i2 == e)[0]
        idx = np.concatenate([sel1, sel2])
        gv = np.concatenate([g1[sel1], g2[sel2]])
        cnt = idx.shape[0]
        assert cnt <= C, f"expert {e} over capacity: {cnt} > {C}"
        idx_per_e.append(idx)

        xg = np.zeros((C, D), np.float32)
        xg[:cnt] = xf[idx]
        xgt = np.ascontiguousarray(
            xg.T.reshape(DC, P, C))               # [dc, p, n]
        w1t = np.ascontiguousarray(
            w1[e].reshape(DC, P, NHG, HG * P).transpose(2, 0, 1, 3))
        w2t = np.ascontiguousarray(
            w2[e].reshape(HC, P, DT, DTS).transpose(2, 0, 1, 3))
        b1t = np.ascontiguousarray(b1[e].reshape(HC, P).T)
        gfull = np.zeros(C, np.float32)
        gfull[:cnt] = gv
        gt = np.ascontiguousarray(gfull.reshape(C // P, P).T)
        in_maps.append(
            {"xgt": xgt, "w1t": w1t, "w2t": w2t, "b1t": b1t, "gt": gt})

    nc = _get_nc()
    res = bass_utils.run_bass_kernel_spmd(
        nc, in_maps, core_ids=list(range(N_CORES)), trace=TRACE)
    global LAST_RESULTS
    LAST_RESULTS = res

    # ---- combine (host): each token occurs in exactly 2 experts, never twice
    # in one, so fancy-index += is safe per expert ----
    out = np.zeros((N, D), np.float32)
    for e in range(E):
        idx = idx_per_e[e]
        out[idx] += res.results[e]["yg"][:idx.shape[0]]

    if np.any(b2):
        gate_full = np.zeros((N, E), np.float32)
        gate_full[np.arange(N), i1] = g1
        gate_full[np.arange(N), i2] = g2
        out += gate_full @ b2.reshape(E, D)

    return out.reshape(B, T, D)

